# revision 42
# baseline (speedup 1.0000x reference)
"""CrossAttention kernel for 8 TRN2 NeuronCores — v3 (ACT/DVE exp split).

Head-parallel sharding (2 heads/core). All-bf16 data path: x/context/weights
arrive as bf16, matmuls run bf16 (1 col/cycle), PSUM accumulation fp32.
Partial outputs are written bf16 and summed (fp32) on the host with the bias.

The v2 kernel was ACT-bound: the softmax exp stream (16.8M elem/core) at
1 elem/lane/cycle @1.2GHz is ~109us alone. v3 offloads a fraction of the
exp tiles to the DVE as a Schraudolph bit-trick exp2 (one tensor_scalar:
i16 = int(s*(SCALE*log2e*128) + (127-c)*128), bitcast bf16), trading ~2%
rms error on those tiles (validated ~8e-3 final metric, gate is 2e-2).
To make DVE room, the per-head K/Q projection drains collapsed from two
[64,512] DVE copies per sub to one [128,512] copy into a natural-layout
tile (GpSimd builds the swapped-half mirror), and a third of the Wo
PSUM->SBUF drains ride ACT as identity-activation copies (same table set
as exp, so no table reload).

S matmuls are emitted as even/odd key-tile pairs at partition bases 0/64 so
they run concurrently in separate PE row groups (K=64 row tiling); K and Q
are mirrored across both partition halves (swapped-half tiles) to enable
this. Engine routing: loads on SP via HWDGE, stores + staging on DVE/GpSimd,
exact exp on ACT, trick exp on DVE.

Measurement note (2026-08-12): the axon-tunneled wall-clock deltas drift
+-40% session-to-session and are superlinear in reps beyond ~10, so only
same-session A/B at reps<=10 is meaningful. Engine-budget numbers from the
static cost model (analyze.py) were the primary optimization signal.
"""
import sys

sys.path.insert(0, "/opt/trn_rl_repo")

import numpy as np
import ml_dtypes
from contextlib import ExitStack

import concourse.bass as bass  # noqa: F401
import concourse.tile as tile
from concourse import bacc, mybir
from concourse.bass_utils import run_bass_kernel_spmd
from concourse.masks import make_identity

B, N, M = 2, 2048, 2048
QDIM = 1024
HEADS = 16
DH = 64
INNER = 1024
NCORES = 8
ES = INNER // NCORES        # 128: e-slice (2 heads * 64) per core
SCALE = DH ** -0.5
T = B * N                   # 4096 tokens total
KC = QDIM // 128            # 8 contraction chunks for projections
MT = M // 128               # 16 key tiles per batch
F32 = mybir.dt.float32
BF = mybir.dt.bfloat16
I16 = mybir.dt.int16
EXP = mybir.ActivationFunctionType.Exp
BF_NP = ml_dtypes.bfloat16
MULT = mybir.AluOpType.mult
ADD = mybir.AluOpType.add

# Schraudolph bit-trick exp2 on DVE: i16 = trunc(s*TRICK_A + TRICK_B) viewed
# as bf16 approximates exp(s*SCALE) (~2% rms on those tiles; the ACT path
# stays exact). TRICK_MC picks which key tiles (mc 0..15) ride the DVE.
LOG2E = 1.4426950408889634
TRICK_C = 0.0430
TRICK_A = float(SCALE * LOG2E * 128.0)
TRICK_B = float((127.0 - TRICK_C) * 128.0)
TRICK_MC = frozenset({4, 6, 9, 11})


def build_nc(reps: int = 1):
    nc = bacc.Bacc("TRN2", target_bir_lowering=False, debug=False,
                   num_devices=NCORES)
    xT = nc.dram_tensor("xT", [QDIM, T], BF, kind="ExternalInput").ap()
    cT = nc.dram_tensor("cT", [QDIM, T], BF, kind="ExternalInput").ap()
    wqT = nc.dram_tensor("wqT", [QDIM, ES], BF, kind="ExternalInput").ap()
    wkT = nc.dram_tensor("wkT", [QDIM, ES], BF, kind="ExternalInput").ap()
    wvT = nc.dram_tensor("wvT", [QDIM, ES], BF, kind="ExternalInput").ap()
    woT = nc.dram_tensor("woT", [ES, QDIM], BF, kind="ExternalInput").ap()
    part = nc.dram_tensor("part", [T, QDIM], BF, kind="ExternalOutput").ap()

    xT3 = xT.rearrange("(kc p) n -> kc p n", p=128)
    cT3 = cT.rearrange("(kc p) n -> kc p n", p=128)

    with tile.TileContext(nc) as tc, ExitStack() as ctx:
        const = ctx.enter_context(tc.tile_pool(name="const", bufs=1))
        big = ctx.enter_context(tc.tile_pool(name="bigsb", bufs=1))
        xsl = ctx.enter_context(tc.tile_pool(name="xsl", bufs=1))
        epool = ctx.enter_context(tc.tile_pool(name="epool", bufs=4))
        bcp = ctx.enter_context(tc.tile_pool(name="bcp", bufs=1))
        outp = ctx.enter_context(tc.tile_pool(name="outp", bufs=4))
        ps_st = ctx.enter_context(tc.tile_pool(name="ps_st", bufs=2, space="PSUM"))
        ps_o = ctx.enter_context(tc.tile_pool(name="ps_o", bufs=1, space="PSUM"))
        pp = ctx.enter_context(tc.tile_pool(name="pp", bufs=2, space="PSUM"))

        # warm the exp table before the first real activation
        warm = const.tile([1, 16], F32)
        nc.vector.memset(warm[:], 0.0)
        warm_o = const.tile([1, 16], F32)
        nc.scalar.activation(warm_o[:], warm[:], EXP)
        wq_sb = const.tile([128, KC, ES], BF)
        wk_sb = const.tile([128, KC, ES], BF)
        wv_sb = const.tile([128, KC, ES], BF)
        wo_sb = const.tile([128, QDIM], BF)
        # K/Q weights first: the first projection matmuls wait on them; wv/wo
        # are issued mid-prologue after the first x/context chunks.
        nc.sync.dma_start(wk_sb[:], wkT.rearrange("(kc p) e -> p kc e", p=128))
        nc.sync.dma_start(wq_sb[:], wqT.rearrange("(kc p) e -> p kc e", p=128))

        pending_tail = None
        for _rep in range(reps):
            # Qn/Kn hold the natural head layout (h0 on partitions 0:64, h1
            # on 64:128); Qm/Km hold the swapped halves (the mirrors the S
            # row-tiling pair trick needs), filled by SBUF->SBUF DMAs.
            Qn = {}
            Qm = {}
            Kn = {}
            Km = {}
            vg = {}
            VT = {}
            oc = {}
            for b in range(B):
                VT[b] = big.tile([128, N], BF, tag=f"vt{b}", name=f"VT{b}_{_rep}")
                oc[b] = big.tile([128, N], BF, tag=f"oc{b}", name=f"oc{b}_{_rep}")
                Qn[b] = big.tile([128, N], BF, tag=f"qn{b}", name=f"Qn{b}_{_rep}")
                Qm[b] = big.tile([128, N], BF, tag=f"qm{b}", name=f"Qm{b}_{_rep}")
                Kn[b] = big.tile([128, N], BF, tag=f"kn{b}", name=f"Kn{b}_{_rep}")
                Km[b] = big.tile([128, N], BF, tag=f"km{b}", name=f"Km{b}_{_rep}")
                # per (key tile, head) V block of 80 cols: 64 V dims + ones
                # col at 64 (softmax denominator row) + pad to 80 for 32B
                # alignment of the XBAR transpose writes
                v = big.tile([128, MT, 2, 80], BF, tag=f"vg{b}",
                             name=f"vg{b}_{_rep}")
                nc.vector.memset(v[:, :, :, DH], 1.0)
                vg[b] = v

            # ---- work units ----------------------------------------------
            def u_load3(src3, col0, width, ndma=2, eng=None):
                """One [128, KC, width] staging tile, loaded by ndma DMAs."""
                t = xsl.tile([128, KC, width], BF,
                             tag=f"xs{width}", bufs=(4 if width == 512 else 3),
                             name=f"xs_{_rep}_{col0}_{width}")
                for d in range(ndma):
                    k0 = d * (KC // ndma)
                    k1 = (d + 1) * (KC // ndma)
                    (eng or nc.sync).dma_start(
                        t[:, k0:k1, :],
                        src3[k0:k1, :, col0:col0 + width].rearrange(
                            "k p n -> p k n"))
                return t

            def u_proj_sub(xs3, cols, w_sb, dsts, dups=(), part=None,
                           dup_eng=None):
                """One 512-token projection accumulation + copies.

                PSUM reads must stay on DVE (GPSIMD cannot access PSUM);
                the partition-swap mirrors are SBUF->SBUF copies on GpSimd
                (SP-DMA mirrors measured no better and risk queue latency
                on the S critical path). Prologue subs pass dup_eng=DVE:
                the first S-pair is gated on those mirrors and the timeline
                shows GpSimd queueing them ~6us behind SWDGE issues while
                DVE idles.
                part=(state, 0|1) splits the contraction into two 0.85us
                emissions so attention S-pairs can slot between them in the
                PE FIFO. Safe because split halves are adjacent in the work
                list and the attention stream never allocates from pp.
                """
                if part is None:
                    kr, do_out = range(KC), True
                    ps = pp.tile([128, 512], F32, tag="x", name=f"ps_{_rep}")
                else:
                    state, half = part
                    if half == 0:
                        state["ps"] = pp.tile([128, 512], F32, tag="x",
                                              name=f"ps_{_rep}")
                        kr, do_out = range(KC // 2), False
                    else:
                        kr, do_out = range(KC // 2, KC), True
                    ps = state["ps"]
                for k in kr:
                    nc.tensor.matmul(ps[:], w_sb[:, k, :], xs3[:, k, cols],
                                     start=(k == 0), stop=(k == KC - 1))
                if do_out:
                    for dst, rows in dsts:
                        nc.vector.tensor_copy(dst, ps[rows, :])
                    for dst, src in dups:
                        if dup_eng is nc.scalar:
                            # prologue only: ACT is idle during the fill and
                            # its stream has nothing queued, so the mirror
                            # lands ~15us earlier than on the busy DVE/Pool
                            # streams (walrus sem thresholds count ALL prior
                            # same-engine completions, not just true deps)
                            nc.scalar.copy(dst, src)
                        elif dup_eng is not None:
                            dup_eng.tensor_copy(dst, src)
                        else:
                            nc.gpsimd.tensor_copy(dst, src)

            def q_dsts(b, col):
                return [(Qn[b][:, col], slice(0, 128))]

            def q_dups(b, col):
                return [
                    (Qm[b][64:128, col], Qn[b][0:64, col]),
                    (Qm[b][0:64, col], Qn[b][64:128, col]),
                ]

            def k_dsts(b, col):
                return [(Kn[b][:, col], slice(0, 128))]

            def k_dups(b, col):
                return [
                    (Km[b][64:128, col], Kn[b][0:64, col]),
                    (Km[b][0:64, col], Kn[b][64:128, col]),
                ]

            def u_vtr(b, g):
                """Transpose one 128-key tile of V (both heads) via XBAR DMA.

                HW XBAR transpose is strictly 2D in[r, c] -> out[c, r] on
                BOTH sides (3D input reads out of range; 3D strided output
                writes garbage -> NaN), so one DMA per (head, key tile).
                """
                for h in range(2):
                    nc.sync.dma_start_transpose(
                        vg[b][:, g, h, 0:DH],
                        VT[b][h * DH:(h + 1) * DH, g * 128:(g + 1) * 128])

            def u_wo(b, nt, tail=False):
                """Output projection for one 128-token tile + store.

                A third of the PSUM->SBUF drain copies ride ACT (identity
                activation, same table set as exp) to balance the ACT/DVE
                pool after the exp offload.
                """
                osb = outp.tile([128, QDIM], BF, tag="os", name=f"os_{_rep}")
                otile = oc[b][:, nt * 128:(nt + 1) * 128]
                on_act = nt % 8 == 2 or (b == 0 and nt % 8 == 3)
                for ob in range(2):
                    po = pp.tile([128, 512], F32, tag="x", name=f"po_{_rep}")
                    nc.tensor.matmul(po[:], otile,
                                     wo_sb[:, ob * 512:(ob + 1) * 512],
                                     start=True, stop=True)
                    dst = osb[:, ob * 512:(ob + 1) * 512]
                    if on_act:
                        nc.scalar.copy(dst, po[:])
                    else:
                        nc.vector.tensor_copy(dst, po[:])
                r0 = b * N + nt * 128
                # tail stores alternate SP/SWDGE so the final 8 stores don't
                # serialize on one queue (timeline showed an 11us SP-only
                # drain); in-flight stores use the gpsimd SWDGE path
                deng = nc.sync if (tail and nt % 2 == 0) else nc.gpsimd
                deng.dma_start(part[r0:r0 + 128, :], osb[:])

            # lazy load groups, issued by explicit schedule entries so the
            # SP queue stays in consumption order
            class Group:
                def __init__(self, src3, b, g):
                    self.args = (src3, b * N + g * 1024)
                    # batch-1 loads ride the gpsimd SWDGE queue so they are
                    # not issue-blocked behind batch-0's SP/HWDGE backlog
                    self.eng = nc.gpsimd if b == 1 else None
                    self.t = None

                def ensure(self):
                    if self.t is None:
                        self.t = u_load3(self.args[0], self.args[1], 1024,
                                         eng=self.eng)
                    return self.t

            grp = {}
            for b in range(B):
                for g in range(2):
                    grp["c", b, g] = Group(cT3, b, g)
                    grp["x", b, g] = Group(xT3, b, g)

            def kv_sub(b, g, half, part=None):
                xs3 = grp["c", b, g].ensure()
                col = slice(g * 1024 + half * 512, g * 1024 + (half + 1) * 512)
                sl = slice(half * 512, (half + 1) * 512)
                u_proj_sub(xs3, sl, wk_sb, k_dsts(b, col), k_dups(b, col),
                           part=part)

            def v_sub(b, g, half, part=None):
                xs3 = grp["c", b, g].ensure()
                col = slice(g * 1024 + half * 512, g * 1024 + (half + 1) * 512)
                sl = slice(half * 512, (half + 1) * 512)
                u_proj_sub(xs3, sl, wv_sb, [(VT[b][:, col], slice(0, 128))],
                           part=part)

            def q_sub(b, g, half, part=None):
                xs3 = grp["x", b, g].ensure()
                col = slice(g * 1024 + half * 512, g * 1024 + (half + 1) * 512)
                sl = slice(half * 512, (half + 1) * 512)
                u_proj_sub(xs3, sl, wq_sb, q_dsts(b, col), q_dups(b, col),
                           part=part)

            # ---- prologue: minimal S(0,0) critical path ------------------
            # Loads are issued first, then the PREVIOUS rep's tail is
            # emitted (its wo/store work runs on the engines while this
            # rep's input DMAs fly), then this rep's projection subs.
            full = slice(0, 512)
            c0a = u_load3(cT3, 0, 512)
            x0a = u_load3(xT3, 0, 512)
            x0b = u_load3(xT3, 512, 512)
            if _rep == 0:
                nc.sync.dma_start(wv_sb[:],
                                  wvT.rearrange("(kc p) e -> p kc e", p=128))
            c0b = u_load3(cT3, 512, 512)
            # prefetch the remaining batch-0 groups eagerly: deadline-paced
            # load units self-delay when the pipeline stalls on them
            grp["c", 0, 1].ensure()
            grp["x", 0, 1].ensure()
            if _rep == 0:
                nc.sync.dma_start(wo_sb[:], woT)
            if pending_tail is not None:
                pending_tail()
                pending_tail = None
            # rep 0 only: ride the idle ACT stream so the first S-pair's
            # mirror wait clears at ~16us instead of ~31us (walrus counter
            # thresholds); later reps' prologues overlap the previous tail
            # where ACT is busy with exps, so GpSimd is better there
            pro_dup = nc.scalar if _rep == 0 else None
            u_proj_sub(c0a, full, wk_sb, k_dsts(0, slice(0, 512)),
                       k_dups(0, slice(0, 512)), dup_eng=pro_dup)
            u_proj_sub(x0a, full, wq_sb, q_dsts(0, slice(0, 512)),
                       q_dups(0, slice(0, 512)), dup_eng=pro_dup)
            u_proj_sub(x0b, full, wq_sb, q_dsts(0, slice(512, 1024)),
                       q_dups(0, slice(512, 1024)), dup_eng=pro_dup)

            # ---- work queue ----------------------------------------------
            work = []

            def add(avail, deadline, cost, fn):
                """avail/deadline in global pair steps (8 phases x 8 pairs).

                Units are force-emitted at their deadline step (consumers
                come >=1 step later, so producers always precede consumers in
                the engine FIFOs); earlier emission happens whenever the
                ACT-slack budget allows. Deadlines spread heavy units evenly.
                (Tried shifting deadlines off phase-start pairs to unclog
                the PE FIFO there — TimelineSim showed the boundary stall is
                the serial S->exp->AV chain at phase end, not emission clog,
                and the shift gained nothing.)
                """
                work.append([avail, deadline, cost, fn])

            def add_load(avail, deadline, key):
                add(avail, deadline, 100, lambda: grp[key].ensure())

            def add_split(avail, d, sub, b, g, half):
                st_ = {}
                add(avail, d - 1, 850,
                    lambda: sub(b, g, half, part=(st_, 0)))
                add(avail, d, 850,
                    lambda: sub(b, g, half, part=(st_, 1)))

            add(0, 0, 1700, lambda: u_proj_sub(
                c0a, full, wv_sb, [(VT[0][:, 0:512], slice(0, 128))]))
            for g in range(4):
                add(0, max(0, g // 2), 100, lambda g=g: u_vtr(0, g))
            add(0, 1, 1700, lambda: u_proj_sub(
                c0b, full, wk_sb, k_dsts(0, slice(512, 1024)),
                k_dups(0, slice(512, 1024))))
            add(0, 2, 1700, lambda: u_proj_sub(
                c0b, full, wv_sb, [(VT[0][:, 512:1024], slice(0, 128))]))
            for g in range(4, 8):
                add(0, g // 2, 100, lambda g=g: u_vtr(0, g))
            add_split(0, 2, v_sub, 0, 1, 0)
            add_split(0, 3, kv_sub, 0, 1, 0)
            for g in range(8, 12):
                add(0, g // 2, 100, lambda g=g: u_vtr(0, g))
            add_split(0, 4, v_sub, 0, 1, 1)
            add_split(0, 5, kv_sub, 0, 1, 1)
            for g in range(12, 16):
                add(0, g // 2, 100, lambda g=g: u_vtr(0, g))
            add_split(2, 11, q_sub, 0, 1, 0)
            add_split(2, 12, q_sub, 0, 1, 1)
            add_load(4, 6, ("c", 1, 0))
            add_split(8, 17, v_sub, 1, 0, 0)
            add_split(8, 18, kv_sub, 1, 0, 0)
            add_split(8, 20, v_sub, 1, 0, 1)
            add_split(8, 21, kv_sub, 1, 0, 1)
            add_load(8, 10, ("c", 1, 1))
            add_split(10, 22, v_sub, 1, 1, 0)
            add_split(10, 23, kv_sub, 1, 1, 0)
            add_split(10, 25, v_sub, 1, 1, 1)
            add_split(10, 26, kv_sub, 1, 1, 1)
            for g in range(16):
                add(16, 26 + g // 4, 100, lambda g=g: u_vtr(1, g))
            add_load(10, 14, ("x", 1, 0))
            add_split(16, 28, q_sub, 1, 0, 0)
            add_split(16, 29, q_sub, 1, 0, 1)
            add_load(16, 28, ("x", 1, 1))
            add_split(24, 38, q_sub, 1, 1, 0)
            add_split(24, 39, q_sub, 1, 1, 1)
            for nt in range(8):
                add(17, 33 + nt, 860, lambda nt=nt: u_wo(0, nt))
            for nt in range(8, 16):
                add(33, 33 + nt, 860, lambda nt=nt: u_wo(0, nt))
            for nt in range(8):
                add(49, 50 + nt, 860, lambda nt=nt: u_wo(1, nt))
            for nt in range(8, 16):
                add(64, 999, 860, lambda nt=nt: u_wo(1, nt, tail=True))

            SLACK_NS = 420.0
            state = {"pair": 0, "budget": 0.0, "debt": 0.0}

            def drain():
                pr = state["pair"]
                state["pair"] += 1
                state["budget"] += SLACK_NS
                while True:
                    pick = None
                    for it in work:
                        if it[1] <= pr:
                            pick = it
                            break
                    if pick is None and state["debt"] < state["budget"]:
                        for it in work:
                            if it[0] <= pr:
                                pick = it
                                break
                    if pick is None:
                        return
                    work.remove(pick)
                    state["debt"] += pick[2]
                    pick[3]()
                    if pick[1] > pr and state["debt"] >= state["budget"]:
                        return

            # ---- unified attention pipeline over 64 pair steps -----------
            PH = [(b, nhf, h) for b in range(B) for nhf in range(2)
                  for h in range(2)]
            oaccs = {}
            es = {}

            def s_pair(ph, pp_):
                b, nhf, h = PH[ph]
                q0 = nhf * 1024
                for sub in range(2):
                    mc = 2 * pp_ + sub
                    rs = slice(0, 64) if sub == 0 else slice(64, 128)
                    # natural tiles hold head h at partitions h*64:(h+1)*64;
                    # the mirror tiles hold it in the other half
                    Kt = Kn[b] if sub == h else Km[b]
                    Qt = Qn[b] if sub == h else Qm[b]
                    st = ps_st.tile([128, 1024], F32, tag="st",
                                    name=f"st_{_rep}")
                    for qb in range(2):
                        nc.tensor.matmul(
                            st[:, qb * 512:(qb + 1) * 512],
                            Kt[rs, mc * 128:(mc + 1) * 128],
                            Qt[rs, q0 + qb * 512:q0 + (qb + 1) * 512],
                            start=True, stop=True)
                    e = epool.tile([128, 1024], BF, tag="e", name=f"e_{_rep}")
                    if mc in TRICK_MC:
                        nc.vector.tensor_scalar(
                            e[:].bitcast(I16), st[:], TRICK_A, TRICK_B,
                            op0=MULT, op1=ADD)
                    else:
                        nc.scalar.activation(e[:], st[:], EXP, scale=SCALE)
                    es[ph, mc] = e

            def av_pair(ph, pp_):
                b, nhf, h = PH[ph]
                oacc = oaccs[ph]
                for sub in range(2):
                    mc = 2 * pp_ + sub
                    e = es.pop((ph, mc))
                    for qb in range(2):
                        nc.tensor.matmul(
                            oacc[0:DH + 1, qb * 512:(qb + 1) * 512],
                            vg[b][:, mc, h, 0:DH + 1],
                            e[:, qb * 512:(qb + 1) * 512],
                            start=(mc == 0), stop=(mc == MT - 1))

            def finalize(ph, tail=False):
                """Free the oacc PSUM slot fast (copy + approx-recip read it
                early), then normalize from the SBUF copy."""
                b, nhf, h = PH[ph]
                q0 = nhf * 1024
                oacc = oaccs.pop(ph)
                rr = bcp.tile([1, 1024], F32, tag="rr", bufs=2,
                              name=f"rr_{_rep}")
                nc.vector.reciprocal(rr[:], oacc[DH:DH + 1, 0:1024])
                osn = bcp.tile([DH, 1024], F32, tag="osn", bufs=2,
                               name=f"osn_{_rep}")
                nc.vector.tensor_copy(osn[:], oacc[0:DH, 0:1024])
                bc = bcp.tile([DH, 1024], F32, tag="bc", bufs=2,
                              name=f"bc_{_rep}")
                nc.gpsimd.partition_broadcast(bc[:], rr[:])
                r0 = h * DH
                eng = nc.vector if tail else nc.gpsimd
                eng.tensor_mul(
                    oc[b][r0:r0 + DH, q0:q0 + 1024],
                    osn[0:DH, :], bc[:])

            for pr in range(64):
                ph, pp_ = divmod(pr, 8)
                if pp_ == 0:
                    oaccs[ph] = ps_o.tile([128, 1024], F32, tag="o",
                                          name=f"oacc_{_rep}")
                s_pair(ph, pp_)
                if pr > 0:
                    pph, ppp = divmod(pr - 1, 8)
                    av_pair(pph, ppp)
                    if ppp == 7:
                        finalize(pph)
                drain()

            def _tail(av_pair=av_pair, finalize=finalize, work=work):
                av_pair(7, 7)
                finalize(7, tail=True)
                for it in work:
                    it[3]()
            pending_tail = _tail
        pending_tail()
    nc.compile()



    return nc


def make_in_maps(x, context, Wq, Wk, Wv, Wo):
    x = np.asarray(x, dtype=np.float32)
    context = np.asarray(context, dtype=np.float32)
    Wq = np.asarray(Wq, dtype=np.float32)
    Wk = np.asarray(Wk, dtype=np.float32)
    Wv = np.asarray(Wv, dtype=np.float32)
    Wo = np.asarray(Wo, dtype=np.float32)
    # single-pass transpose+cast (astype materializes the copy)
    xT = x.reshape(T, QDIM).T.astype(BF_NP)
    cT = context.reshape(T, QDIM).T.astype(BF_NP)
    in_maps = []
    for c in range(NCORES):
        es = slice(c * ES, (c + 1) * ES)
        in_maps.append({
            "xT": xT,
            "cT": cT,
            "wqT": np.ascontiguousarray(Wq[es, :].T).astype(BF_NP),
            "wkT": np.ascontiguousarray(Wk[es, :].T).astype(BF_NP),
            "wvT": np.ascontiguousarray(Wv[es, :].T).astype(BF_NP),
            "woT": np.ascontiguousarray(Wo[:, es].T).astype(BF_NP),
        })
    return in_maps


_NC_CACHE = {}


def get_nc(reps: int = 1):
    if reps not in _NC_CACHE:
        _NC_CACHE[reps] = build_nc(reps)
    return _NC_CACHE[reps]


def run_on_hw(in_maps, reps: int = 1):
    nc = get_nc(reps)
    return run_bass_kernel_spmd(nc, in_maps, core_ids=list(range(NCORES)))


def kernel(x, context, Wq, Wk, Wv, Wo, bo):
    in_maps = make_in_maps(x, context, Wq, Wk, Wv, Wo)
    res = run_on_hw(in_maps, reps=1)
    # fused host reduction: accumulate bf16 partials into one f32 buffer
    # in-place (no per-core astype temporaries)
    acc = np.broadcast_to(np.asarray(bo, dtype=np.float32)[None, :],
                          (T, QDIM)).copy()
    for i in range(NCORES):
        np.add(acc, res.results[i]["part"], out=acc)
    return acc.reshape(B, N, QDIM)

